# revision 1
# baseline (speedup 1.0000x reference)
"""LossVariance segment-reduce kernel for 8x Trainium2 NeuronCores.

Strategy: data-parallel over batch B=8 (one sample per core). Per core:
  - pixels laid out as [128, 8192] (partition-contiguous)
  - per 128-pixel chunk: build a one-hot [128 px, 512 labels] via
    tensor_scalar(is_equal) against a prebuilt iota row, then PE matmul
    lhsT=[v1|v2|1] (stats, bf16) x rhs=one-hot accumulating into a
    PSUM [3, 512] bin table (s, ss, cnt per label).
  - epilogue computes the per-sample loss scalar on-device.
Host averages the 8 per-core scalars.
"""

import sys

sys.path.insert(0, "/opt/trn_rl_repo")

import numpy as np

import concourse.bacc as bacc
import concourse.mybir as mybir
from concourse import bass_utils
from concourse.tile import TileContext

B = 8
C = 3
H = W = 1024
P = H * W           # pixels per sample
NPART = 128
FTOT = P // NPART   # 8192 columns
FB = 2048           # columns per pipeline block
NBLK = FTOT // FB
L = 512             # padded label count (labels 0..499 used)

_CACHE = {}


def _build():
    nc = bacc.Bacc("TRN2", target_bir_lowering=False, debug=False, num_devices=B)
    f32 = mybir.dt.float32
    bf16 = mybir.dt.bfloat16
    i32 = mybir.dt.int32

    x_d = nc.dram_tensor("xc", [C, P], f32, kind="ExternalInput")
    t_d = nc.dram_tensor("tc", [P], i32, kind="ExternalInput")
    loss_d = nc.dram_tensor("loss", [1], f32, kind="ExternalOutput")

    xv = x_d.ap().rearrange("c (p f) -> c p f", p=NPART)   # [3, 128, 8192]
    tv = t_d.ap().rearrange("(p f) -> p f", p=NPART)       # [128, 8192]

    with TileContext(nc) as tc:
        with (
            tc.tile_pool(name="const", bufs=1) as cpool,
            tc.tile_pool(name="xin", bufs=2) as xpool,
            tc.tile_pool(name="work", bufs=2) as wpool,
            tc.tile_pool(name="oh", bufs=8) as ohpool,
            tc.tile_pool(name="psum", bufs=1, space="PSUM") as ppool,
            tc.tile_pool(name="epi", bufs=1) as epool,
        ):
            # iota row 0..511 on every partition, fp32
            iota_i = cpool.tile([NPART, L], i32)
            nc.gpsimd.iota(iota_i[:], pattern=[[1, L]], base=0, channel_multiplier=0)
            iota_f = cpool.tile([NPART, L], f32)
            nc.vector.tensor_copy(iota_f[:], iota_i[:])

            acc = ppool.tile([C, L], f32, space="PSUM")  # rows: s, ss, cnt

            nchunks = 0
            for blk in range(NBLK):
                sl = slice(blk * FB, (blk + 1) * FB)
                x0 = xpool.tile([NPART, FB], f32, tag="x0")
                x1 = xpool.tile([NPART, FB], f32, tag="x1")
                x2 = xpool.tile([NPART, FB], f32, tag="x2")
                ti = xpool.tile([NPART, FB], i32, tag="ti")
                nc.sync.dma_start(x0[:], xv[0, :, sl])
                nc.sync.dma_start(x1[:], xv[1, :, sl])
                nc.sync.dma_start(x2[:], xv[2, :, sl])
                nc.sync.dma_start(ti[:], tv[:, sl])

                tf = wpool.tile([NPART, FB], f32, tag="tf")
                nc.vector.tensor_copy(tf[:], ti[:])

                V = wpool.tile([NPART, 3, FB], bf16, tag="V")
                ta = wpool.tile([NPART, FB], f32, tag="ta")
                tb = wpool.tile([NPART, FB], f32, tag="tb")
                # v1 = x0+x1+x2 (bf16 out)
                nc.vector.tensor_add(ta[:], x0[:], x1[:])
                nc.vector.tensor_add(V[:, 0, :], ta[:], x2[:])
                # v2 = x0^2+x1^2+x2^2 (squares on ACT, adds on DVE)
                nc.scalar.square(ta[:], x0[:])
                nc.scalar.square(tb[:], x1[:])
                nc.vector.tensor_add(ta[:], ta[:], tb[:])
                nc.scalar.square(tb[:], x2[:])
                nc.vector.tensor_add(V[:, 1, :], ta[:], tb[:])
                nc.vector.memset(V[:, 2, :], 1.0)

                for q in range(FB):
                    oh = ohpool.tile([NPART, L], bf16, tag="oh")
                    nc.vector.tensor_scalar(
                        oh[:], iota_f[:], tf[:, q : q + 1], None,
                        mybir.AluOpType.is_equal,
                    )
                    nc.tensor.matmul(
                        out=acc[:],
                        lhsT=V[:, :, q],
                        rhs=oh[:],
                        start=(nchunks == 0),
                        stop=(nchunks == FTOT - 1),
                    )
                    nchunks += 1

            # ---- epilogue: per-sample loss from [3, 512] bins ----
            stats = epool.tile([C, L], f32)
            nc.vector.tensor_copy(stats[:], acc[:])
            s_r = stats[0:1, :]
            ss_r = epool.tile([1, L], f32)
            cnt_r = epool.tile([1, L], f32)
            # move rows 1,2 onto partition 0 via SBUF->SBUF DMA
            nc.sync.dma_start(ss_r[:], stats[1:2, :])
            nc.sync.dma_start(cnt_r[:], stats[2:3, :])

            lmask = epool.tile([1, L], f32)
            nc.vector.memset(lmask[:], 1.0)
            nc.vector.memset(lmask[0:1, 0:1], 0.0)
            nc.vector.memset(lmask[0:1, 500:L], 0.0)

            op = mybir.AluOpType
            ea = epool.tile([1, L], f32)
            eb = epool.tile([1, L], f32)
            ec = epool.tile([1, L], f32)
            ed = epool.tile([1, L], f32)
            # ea = N = 3*cnt
            nc.vector.tensor_scalar(ea[:], cnt_r[:], 3.0, None, op.mult)
            # ec = 1/max(N,1)
            nc.vector.tensor_scalar(eb[:], ea[:], 1.0, None, op.max)
            nc.vector.reciprocal(ec[:], eb[:])
            # eb = s*s * ec
            nc.vector.tensor_mul(eb[:], s_r, s_r)
            nc.vector.tensor_mul(eb[:], eb[:], ec[:])
            # ec = ss - eb
            nc.vector.tensor_tensor(ec[:], ss_r[:], eb[:], op.subtract)
            # eb = 1/max(N-1,1)
            nc.vector.tensor_scalar(ea[:], ea[:], -1.0, None, op.add)
            nc.vector.tensor_scalar(ea[:], ea[:], 1.0, None, op.max)
            nc.vector.reciprocal(eb[:], ea[:])
            # ed = var = ec*eb * (cnt>1) * lmask
            nc.vector.tensor_mul(ed[:], ec[:], eb[:])
            nc.vector.tensor_scalar(ea[:], cnt_r[:], 1.0, None, op.is_gt)
            nc.vector.tensor_mul(ed[:], ed[:], ea[:])
            nc.vector.tensor_mul(ed[:], ed[:], lmask[:])
            # ea = present = (cnt>0) * lmask
            nc.vector.tensor_scalar(ea[:], cnt_r[:], 0.0, None, op.is_gt)
            nc.vector.tensor_mul(ea[:], ea[:], lmask[:])

            nu = epool.tile([1, 1], f32)
            nc.vector.tensor_reduce(nu[:], ea[:], mybir.AxisListType.X, op.add)
            vs = epool.tile([1, 1], f32)
            nc.vector.tensor_reduce(vs[:], ed[:], mybir.AxisListType.X, op.add)
            nue = epool.tile([1, 1], f32)
            nc.vector.tensor_scalar(nue[:], nu[:], 1e-8, None, op.add)
            rnu = epool.tile([1, 1], f32)
            nc.vector.reciprocal(rnu[:], nue[:])
            res = epool.tile([1, 1], f32)
            nc.vector.tensor_mul(res[:], vs[:], rnu[:])
            nc.sync.dma_start(loss_d.ap().rearrange("(p x) -> p x", p=1), res[:])

    nc.compile()
    return nc


def _get_nc():
    if "nc" not in _CACHE:
        _CACHE["nc"] = _build()
    return _CACHE["nc"]


def _in_maps(x: np.ndarray, target: np.ndarray):
    in_maps = []
    for b in range(B):
        in_maps.append({
            "xc": np.ascontiguousarray(x[b].reshape(C, P), dtype=np.float32),
            "tc": np.ascontiguousarray(target[b].reshape(P), dtype=np.int32),
        })
    return in_maps


def kernel(x: np.ndarray, target: np.ndarray) -> np.ndarray:
    nc = _get_nc()
    res = bass_utils.run_bass_kernel_spmd(nc, _in_maps(x, target), core_ids=list(range(B)))
    vals = [float(res.results[b]["loss"][0]) for b in range(B)]
    return np.float32(sum(vals) / B)

